# revision 18
# baseline (speedup 1.0000x reference)
"""Chunked cross-attention (RETRO-style) Trainium2 Bass kernel.

Contract: kernel(**inputs) takes FULL unsharded inputs (as produced by the
problem's setup_inputs) and returns the FULL [4, 2048, 1024] f32 output.

Sharding: data-parallel over (batch, chunk-half). Core i handles batch i//2,
chunks [16*(i%2), 16*(i%2)+16). Each core is fully independent (no
collectives). Host folds ln_g/ln_b into Wq/bq, casts e to fp8 (packed as
2-byte pairs so the DMA-transpose XBAR delivers DoubleRow-ready e^T without
any on-chip casts; Wq/Wk/Wv rows are permuted to the matching pair order),
slices h/e per core, and stitches the 8 per-core outputs back together.

Per core the kernel runs 8 iterations of 2 chunks (128 query tokens, 512 kv
tokens) each:
  LN(h) -> x_hat (fp8) -> pair-transpose -> q^T = Wq^T @ x_hat^T
  e^T (pair-transposed by DMA) -> k^T = Wk^T @ e^T ; v = e @ Wv (with an
  appended ones-column per head so attn@v also produces the softmax
  denominator Z per query)
  per head: scores^T = k^T_blk^T... computed directly in [kv, q] layout
  (lhsT = k^T slice, rhs = q^T slice), exp (ScalarE, unnormalized),
  out = exp^T @ [v | 1] -> [q, dk | Z] with q on partitions, then the
  normalize is a cheap per-partition scale during the PSUM drain; one PE
  transpose per head-pair re-orients out^T for the Wo GEMM.
Matmuls run in bf16/fp8 with f32 PSUM accumulation; LN + softmax stats f32.
"""

import os
import sys

sys.path.insert(0, "/opt/trn_rl_repo")

from contextlib import ExitStack

import numpy as np
import ml_dtypes

import concourse.bass as bass
import concourse.bacc as bacc
import concourse.mybir as mybir
import concourse.tile as tile
from concourse.bass_utils import run_bass_kernel_spmd
from concourse.masks import make_identity

P = 128
D = 1024
H = 16
DK = 64
L = 64
ITERS = 8  # 2 chunks per iteration, 16 chunks per core
EPS = 1e-5
SCALE = 1.0 / 8.0  # 1/sqrt(DK)
WARMUP = 0

F32 = mybir.dt.float32
BF16 = mybir.dt.bfloat16
FP8 = mybir.dt.float8e4
F8 = ml_dtypes.float8_e4m3
VSCALE = 64.0  # Wv/Wk/Wq/Wo are pre-scaled by this on host (fp8 subnormal dodge)
BF = ml_dtypes.bfloat16

LAST_EXEC_NS = None
LAST_RESULTS = None


def build_nc(with_bq, with_bk, with_bv, with_bo):
    nc = bacc.Bacc("TRN2", target_bir_lowering=False, debug=False)

    h_s = nc.dram_tensor("h_s", [ITERS * P, D], F32, kind="ExternalInput")
    e_s = nc.dram_tensor("e_s", [ITERS * 512, D], BF16, kind="ExternalInput")
    wq_d = nc.dram_tensor("wq", [D, D], FP8, kind="ExternalInput")
    wk_d = nc.dram_tensor("wk", [D, D], FP8, kind="ExternalInput")
    wv_d = nc.dram_tensor("wv", [D, D], FP8, kind="ExternalInput")
    wo_d = nc.dram_tensor("wo", [D, D], FP8, kind="ExternalInput")
    bq_d = nc.dram_tensor("bq", [1, D], F32, kind="ExternalInput")
    bk_d = nc.dram_tensor("bk", [1, D], F32, kind="ExternalInput")
    bv_d = nc.dram_tensor("bv", [1, D], F32, kind="ExternalInput")
    bo_d = nc.dram_tensor("bo", [1, D], F32, kind="ExternalInput")
    out_s = nc.dram_tensor("out_s", [ITERS * P, D], F32, kind="ExternalOutput")

    Exp = mybir.ActivationFunctionType.Exp
    Square = mybir.ActivationFunctionType.Square
    Ident = mybir.ActivationFunctionType.Identity
    X = mybir.AxisListType.X

    with tile.TileContext(nc) as tc, ExitStack() as ctx:
        consts = ctx.enter_context(tc.tile_pool(name="consts", bufs=1))
        ident = consts.tile([P, P], BF16)
        make_identity(nc, ident)
        ones = consts.tile([1, 512], F32)
        nc.vector.memset(ones, 1.0)

        # weight tiles (DMAs emitted below in consumer-priority order)
        wk_t = consts.tile([P, 4, 2, D], FP8)
        wq_t = consts.tile([P, 4, 2, D], FP8)
        wv_t = consts.tile([P, 4, 2, D], FP8)
        wo_t = consts.tile([P, 4, 2, D], FP8)

        bq_t = bk_t = bv_t = bo_t = None
        if with_bq:
            bq_t = consts.tile([1, D], F32, name="bq_t")
            nc.sync.dma_start(bq_t, bq_d)
        if with_bk:
            bk_t = consts.tile([1, D], F32, name="bk_t")
            nc.sync.dma_start(bk_t, bk_d)
        if with_bv:
            bv_t = consts.tile([1, D], F32, name="bv_t")
            nc.sync.dma_start(bv_t, bv_d)
        if with_bo:
            bo_t = consts.tile([1, D], F32, name="bo_t")
            nc.sync.dma_start(bo_t, bo_d)

        res = ctx.enter_context(tc.tile_pool(name="res", bufs=1))
        sb = ctx.enter_context(tc.tile_pool(name="sb", bufs=2))
        hd = ctx.enter_context(tc.tile_pool(name="hd", bufs=8))
        psB = ctx.enter_context(tc.tile_pool(name="psB", bufs=4, space="PSUM"))
        # per-head attention bank: cols 0:256 scores, 256:321 out|Z,
        # f32-words 384:448 = bf16 region for the pair's out-transpose
        psAtt = ctx.enter_context(tc.tile_pool(name="psAtt", bufs=4,
                                               space="PSUM"))

        # PE warmup: dummy matmuls so HAM un-throttles the clock before the
        # real work arrives (the initial DMA wait would otherwise be cold).
        if WARMUP:
            warm = consts.tile([P, 512], BF16, name="warm")
            nc.vector.memset(warm, 0.0)
            wp = psB.tile([P, 512], F32, name="m")
            for i in range(WARMUP):
                nc.tensor.matmul(wp, warm[:, 0:P], warm, start=(i == 0),
                                 stop=(i == WARMUP - 1))
            warm_out = consts.tile([P, 512], BF16, name="warm_out")
            nc.vector.tensor_copy(warm_out, wp)

        hx_all = res.tile([P, ITERS, D], F32)
        stats = res.tile([P, ITERS, 8], F32)
        qT_all = res.tile([P, ITERS, 8, P], BF16)

        # DMA emission order = scheduler priority. Get iter-0's operands in
        # first (eT0 + wk -> k^T, h0 + wq -> LN -> q^T, wv -> v), then the
        # bulk loads.
        eTs = [sb.tile([P, 8, 512], BF16, name="eT") for _ in range(ITERS)]
        nc.sync.dma_start(eTs[0], e_s[0:512, :], transpose=True)
        nc.sync.dma_start(wk_t, wk_d.rearrange("(kp h p) m -> p kp h m", p=P, h=2))
        nc.sync.dma_start(hx_all[:, 0, :], h_s[0:P, :])
        nc.sync.dma_start(wq_t, wq_d.rearrange("(kp h p) m -> p kp h m", p=P, h=2))
        nc.sync.dma_start(wv_t, wv_d.rearrange("(kp h p) m -> p kp h m", p=P, h=2))
        for it in range(1, ITERS):
            nc.sync.dma_start(hx_all[:, it, :], h_s[it * P:(it + 1) * P, :])
        nc.sync.dma_start(wo_t, wo_d.rearrange("(kp h p) m -> p kp h m", p=P, h=2))
        nc.sync.dma_start(eTs[1], e_s[512:1024, :], transpose=True)

        def dr_view(t):
            # bf16-carrier [P, g, S] -> fp8 DoubleRow view [P, g, 2, S]
            return t.bitcast(FP8).rearrange("p g (s r) -> p g r s", r=2)

        # ===== prologue: LN + q^T for all 8 iterations =====
        for it in range(ITERS):
            hx = hx_all[:, it, :]
            ssum = stats[:, it, 0:1]
            ssq = stats[:, it, 1:2]
            negmu = stats[:, it, 2:3]
            musq = stats[:, it, 3:4]
            var = stats[:, it, 4:5]
            nc.vector.reduce_sum(ssum, hx, axis=X)
            sqscr = sb.tile([P, D], BF16, name="sqscr")
            nc.scalar.activation(sqscr, hx, Square, accum_out=ssq)
            nc.vector.tensor_scalar_mul(negmu, ssum, -1.0 / D)
            nc.vector.tensor_mul(musq, negmu, negmu)
            nc.vector.tensor_scalar(var, ssq, 1.0 / D, EPS,
                                    op0=mybir.AluOpType.mult,
                                    op1=mybir.AluOpType.add)
            nc.vector.tensor_sub(var, var, musq)
        # batched sqrt (few ACT table loads) + reciprocal, split so the
        # first iterations' x_hat unblocks before all stats are in
        nc.scalar.sqrt(stats[:, 0:2, 5:6], stats[:, 0:2, 4:5])
        nc.vector.reciprocal(stats[:, 0:2, 6:7], stats[:, 0:2, 5:6])
        nc.scalar.sqrt(stats[:, 2:, 5:6], stats[:, 2:, 4:5])
        nc.vector.reciprocal(stats[:, 2:, 6:7], stats[:, 2:, 5:6])
        def emit_prologue_iter(it):
            hx = hx_all[:, it, :]
            negmu = stats[:, it, 2:3]
            rstd = stats[:, it, 6:7]
            xh = sb.tile([P, D], FP8, name="xh")
            nc.vector.tensor_scalar(xh, hx, negmu, rstd,
                                    op0=mybir.AluOpType.add,
                                    op1=mybir.AluOpType.mult)
            xT = sb.tile([P, 4, P], BF16, name="xT")
            nc.sync.dma_start(xT, xh.bitcast(BF16), transpose=True)
            xT8 = dr_view(xT)
            for m in range(8):
                pq = psAtt.tile([P, 512], F32, name="s")[:, 0:P]
                for kp in range(4):
                    nc.tensor.matmul(pq, wq_t[:, kp, :, m * P:(m + 1) * P],
                                     xT8[:, kp, :, :],
                                     start=(kp == 0),
                                     stop=(kp == 3 and not with_bq),
                                     perf_mode=mybir.MatmulPerfMode.DoubleRow)
                if with_bq:
                    nc.tensor.matmul(pq, bq_t[0:1, m * P:(m + 1) * P],
                                     ones[0:1, 0:P], start=False, stop=True)
                nc.scalar.mul(qT_all[:, it, m, :], pq, 1.0 / VSCALE)

        # ===== main loop, software-pipelined emission =====
        # Emission order drives the Tile scheduler's priorities. Interleaving
        # iteration it+1's projection groups between iteration it's head
        # pairs keeps the PE streaming big matmuls while ScalarE/VectorE
        # chew on the softmax chain.
        kTs = {}
        vsxs = {}
        eT8s = {}

        def emit_proj_part(it, part):
            """part 0-7: k^T m-tile; part 8-15: v (t, nh) tile."""
            eT = eTs[it]
            if part == 0:
                kTs[it] = sb.tile([P, 8, 512], BF16, name="kT")
                vsxs[it] = sb.tile([P, 4, H, 66], BF16, name="vsx")
                # ones column for the softmax-denominator trick
                ocol = vsxs[it].rearrange("p t h x -> p (t h) x")[:, :, 64:65]
                nc.vector.memset(ocol, 1.0)
                eT8s[it] = sb.tile([P, 4, 2, 512], FP8, name="eT8")
                for kp in range(4):
                    for hh in range(2):
                        nc.vector.tensor_copy(eT8s[it][:, kp, hh, :],
                                              eT[:, 2 * kp + hh, :])
            eT8 = eT8s[it]
            if part < 8:
                m = part
                pk = psB.tile([P, 512], F32, name="m")
                for kp in range(4):
                    nc.tensor.matmul(pk, wk_t[:, kp, :, m * P:(m + 1) * P],
                                     eT8[:, kp, :, :],
                                     start=(kp == 0),
                                     stop=(kp == 3 and not with_bk),
                                     perf_mode=mybir.MatmulPerfMode.DoubleRow)
                if with_bk:
                    nc.tensor.matmul(pk, bk_t[0:1, m * P:(m + 1) * P],
                                     ones[0:1, 0:512], start=False, stop=True)
                nc.vector.tensor_scalar_mul(kTs[it][:, m, :], pk, 1.0 / VSCALE)
            else:
                t, nh = divmod(part - 8, 2)
                pv = psB.tile([P, 512], F32, name="m")
                for kp in range(4):
                    nc.tensor.matmul(pv, eT8[:, kp, :, t * P:(t + 1) * P],
                                     wv_t[:, kp, :, nh * 512:(nh + 1) * 512],
                                     start=(kp == 0),
                                     stop=(kp == 3 and not with_bv),
                                     perf_mode=mybir.MatmulPerfMode.DoubleRow)
                if with_bv:
                    nc.tensor.matmul(pv, ones[0:1, 0:P],
                                     bv_t[0:1, nh * 512:(nh + 1) * 512],
                                     start=False, stop=True)
                dst = vsxs[it][:, t, nh * 8:(nh + 1) * 8, 0:64]
                if nh == 0:
                    nc.vector.tensor_scalar_mul(dst, pv, 1.0 / VSCALE)
                else:
                    nc.scalar.mul(dst, pv, 1.0 / VSCALE)

        # interleave the LN/Q prologue with iteration 0's projections so the
        # PE streams K/V matmuls while VectorE/DMA work through the LN chain
        for it in range(ITERS):
            emit_prologue_iter(it)
            emit_proj_part(0, 2 * it)
            emit_proj_part(0, 2 * it + 1)

        for it in range(ITERS):
            if it + 2 < ITERS:
                nc.sync.dma_start(eTs[it + 2],
                                  e_s[(it + 2) * 512:(it + 3) * 512, :],
                                  transpose=True)
            kT = kTs[it]
            vsx = vsxs[it]
            oT = sb.tile([P, 4, 2, P], FP8, name="oT")
            for hp in range(8):
                if it + 1 < ITERS:
                    emit_proj_part(it + 1, 2 * hp)
                    emit_proj_part(it + 1, 2 * hp + 1)
                rz = hd.tile([P, 2], F32, name="rz")
                posb = hd.tile([P, P], BF16, name="posb")
                banks = []
                for ph in range(2):
                    h_ = 2 * hp + ph
                    A = psAtt.tile([P, 512], F32, name="s")
                    banks.append(A)
                    # scores^T in [kv, q] layout: lhsT = k^T slice (dk on
                    # partitions), rhs = q^T slice. 4 blocks (chunk, kv-half)
                    for c in range(2):
                        for u in range(2):
                            nc.tensor.matmul(
                                A[:, (2 * c + u) * 64:(2 * c + u + 1) * 64],
                                kT[ph * 64:(ph + 1) * 64, hp,
                                   c * 256 + u * P:c * 256 + (u + 1) * P],
                                qT_all[ph * 64:(ph + 1) * 64, it, hp,
                                       c * 64:(c + 1) * 64],
                                start=True, stop=True)
                    # softmax: exp only; no max-sub needed (|scores/8| is a
                    # few units at most), Z comes from the ones-column below
                    ex = hd.tile([P, 256], BF16, name="ex")
                    nc.scalar.activation(ex[:, 0:128], A[:, 0:128], Exp,
                                         scale=SCALE)
                    nc.scalar.activation(ex[:, 128:256], A[:, 128:256], Exp,
                                         scale=SCALE)
                    # out = exp^T @ [v | 1] -> [q, dk | Z], q on partitions
                    for c in range(2):
                        for u in range(2):
                            nc.tensor.matmul(
                                A[c * 64:(c + 1) * 64, 256:321],
                                ex[:, (2 * c + u) * 64:(2 * c + u + 1) * 64],
                                vsx[:, 2 * c + u, h_, 0:65],
                                start=(u == 0), stop=(u == 1))
                # normalize during drain: per-partition scale by 1/Z
                nc.vector.reciprocal(rz[:, 0:1], banks[0][:, 320:321])
                nc.vector.reciprocal(rz[:, 1:2], banks[1][:, 320:321])
                nc.vector.tensor_scalar_mul(posb[:, 0:64],
                                            banks[0][:, 256:320], rz[:, 0:1])
                nc.scalar.activation(posb[:, 64:128], banks[1][:, 256:320],
                                     Ident, scale=rz[:, 1:2])
                # re-orient for the Wo GEMM: [q, 2*dk] -> [2*dk, q]
                pt = banks[1].bitcast(BF16)[:, 768:896]
                nc.tensor.transpose(pt, posb, ident)
                if hp % 2 == 0:
                    nc.vector.tensor_copy(oT[:, hp // 2, hp % 2, :], pt)
                else:
                    nc.scalar.copy(oT[:, hp // 2, hp % 2, :], pt)

            # ---- final: out = oT.T @ Wo (+bo) + h ----
            outsb = sb.tile([P, D], F32, name="outsb")
            for nh in range(2):
                pf = psB.tile([P, 512], F32, name="m")
                for kp in range(4):
                    nc.tensor.matmul(pf, oT[:, kp, :, :],
                                     wo_t[:, kp, :, nh * 512:(nh + 1) * 512],
                                     start=(kp == 0),
                                     stop=(kp == 3 and not with_bo),
                                     perf_mode=mybir.MatmulPerfMode.DoubleRow)
                if with_bo:
                    nc.tensor.matmul(pf, ones[0:1, 0:P],
                                     bo_t[0:1, nh * 512:(nh + 1) * 512],
                                     start=False, stop=True)
                nc.vector.scalar_tensor_tensor(
                    outsb[:, nh * 512:(nh + 1) * 512], pf, 1.0 / VSCALE,
                    hx_all[:, it, nh * 512:(nh + 1) * 512],
                    op0=mybir.AluOpType.mult, op1=mybir.AluOpType.add)
            nc.sync.dma_start(out_s[it * P:(it + 1) * P, :], outsb)

    nc.compile()
    return nc


def _pair_perm(w):
    """Permute D rows so host row (g*256 + r*128 + p) holds original row
    (g*256 + 2p + r) — matches the pair order the 2-byte DMA transpose
    produces for the fp8-packed activations."""
    return np.ascontiguousarray(
        w.reshape(4, 128, 2, w.shape[1]).transpose(0, 2, 1, 3).reshape(w.shape))


def make_in_maps(h, e, Wq, bq, Wk, bk, Wv, bv, Wo, bo, ln_g, ln_b):
    """Shard/cast host-side. Returns (in_maps, bias_flags)."""
    h = np.asarray(h, dtype=np.float32)
    e = np.asarray(e, dtype=np.float32)
    Wq = np.asarray(Wq, dtype=np.float32)
    Wk = np.asarray(Wk, dtype=np.float32)
    Wv = np.asarray(Wv, dtype=np.float32)
    Wo = np.asarray(Wo, dtype=np.float32)
    bq = np.asarray(bq, dtype=np.float32)
    bk = np.asarray(bk, dtype=np.float32)
    bv = np.asarray(bv, dtype=np.float32)
    bo = np.asarray(bo, dtype=np.float32)
    ln_g = np.asarray(ln_g, dtype=np.float32)
    ln_b = np.asarray(ln_b, dtype=np.float32)

    # Fold LN affine into the Q projection: q = x_hat@(g*Wq) + (b@Wq + bq)
    wq_eff = _pair_perm((ln_g[:, None] * Wq * 64.0)).astype(F8)
    bq_eff = (ln_b @ Wq + bq).astype(np.float32)[None, :]
    wk_b = (Wk * 64.0).astype(F8)
    wv_b = (Wv * 64.0).astype(F8)
    wo_b = (Wo * 64.0).astype(F8)

    flags = (bool(np.any(bq_eff)), bool(np.any(bk)), bool(np.any(bv)),
             bool(np.any(bo)))

    B, S, _ = h.shape
    in_maps = []
    for core in range(8):
        b, half = divmod(core, 2)
        s0 = 1024 * half + (L - 1)
        h_sh = np.zeros((1024, D), np.float32)
        n = min(1024, S - s0)
        h_sh[:n] = h[b, s0:s0 + n]
        e_sh = np.ascontiguousarray(
            e[b, 16 * half:16 * half + 16].reshape(4096, D)).astype(BF)
        in_maps.append({
            "h_s": h_sh,
            "e_s": e_sh,
            "wq": wq_eff, "wk": wk_b, "wv": wv_b, "wo": wo_b,
            "bq": bq_eff * 64.0, "bk": bk[None, :] * 64.0, "bv": bv[None, :] * 64.0,
            "bo": bo[None, :] * 64.0,
        })
    return in_maps, flags


def assemble(h, results):
    h = np.asarray(h, dtype=np.float32)
    out = np.empty_like(h)
    out[:, :L - 1] = h[:, :L - 1]
    for core in range(8):
        b, half = divmod(core, 2)
        shard = results[core]["out_s"]
        s0 = 1024 * half + (L - 1)
        n = min(1024, 2048 - s0)
        out[b, s0:s0 + n] = shard[:n]
    return out


def _enable_axon_trace():
    """The image lacks antenv.axon_hooks; synthesize it with the ctypes NTFF
    hook from trn_boot so run_bass_kernel_spmd(trace=True) works, and no-op
    the S3 artifact upload."""
    import types

    try:
        import antenv.axon_hooks  # noqa: F401
        have = True
    except ImportError:
        have = False
    if not have:
        if "/root/.axon_site" not in sys.path:
            sys.path.insert(0, "/root/.axon_site")
        from trn_agent_boot.trn_boot import _ntff_profile_via_ctypes

        hook = _ntff_profile_via_ctypes("/opt/axon/libaxon_pjrt.so")
        mod = types.ModuleType("antenv.axon_hooks")
        mod._hook = hook
        mod.get_axon_ntff_profile_hook = lambda: mod._hook
        mod.set_axon_ntff_profile_hook = lambda h: setattr(mod, "_hook", h)
        sys.modules["antenv.axon_hooks"] = mod
        import antenv
        antenv.axon_hooks = mod
    import concourse.bass_utils as bu
    bu.upload_artifacts = lambda tmpdir: "local://" + tmpdir


def kernel(**inputs):
    global LAST_EXEC_NS, LAST_RESULTS
    in_maps, flags = make_in_maps(**inputs)
    nc = build_nc(*flags)
    trace = bool(int(os.environ.get("KBENCH_TRACE", "0")))
    if trace:
        try:
            _enable_axon_trace()
        except Exception as exc:  # profiling is best-effort
            print(f"trace setup failed ({exc!r}); running untraced")
            trace = False
    res = run_bass_kernel_spmd(nc, in_maps, core_ids=list(range(8)),
                               trace=trace)
    LAST_EXEC_NS = res.exec_time_ns
    LAST_RESULTS = res
    return assemble(inputs["h"], res.results)
